# revision 1
# baseline (speedup 1.0000x reference)
"""Causal self-attention Trainium2 kernel (8 NeuronCores, batch x head-group sharded).

Problem: hidden [2, 2048, 1024], 16 heads x 64 dim, causal softmax attention,
QKV projection + output projection, all fp32.

Sharding: core c handles batch b = c//4 and head group g = c%4 (4 heads).
Each core computes qkv projections for its 4 heads, causal attention, and a
partial output projection (row-shard of Wo). Host sums the 4 partials per batch
and adds bo.

Matmuls run as float32r (TF32-like rounding, ~2e-4 rel err, full PE rate at
free-dim >= 256). Layouts chosen so no transposes are ever needed on device:
  - hidden is transposed on host once -> hT [1024, 2048] per batch
  - q,k are produced feature-major (qT/kT [feat, seq]); v is seq-major
  - scores are computed transposed (kT as weights): scoresT [j, i]
  - PV uses [v | ones] weights -> psum rows 0:64 = attn out^T, row 64 = denom

The per-seq-chunk pipeline interleaves the QKV projection of chunk c+1 with
the attention of chunk c so PE work overlaps the ACT-bound softmax.
"""
import math
import os
import re as _re

import numpy as np

import concourse.bass as bass
import concourse.mybir as mybir
import concourse.tile as tile

# ---------------------------------------------------------------------------
# The final TileContext drain carries one wait per proc (the Drain instruction
# has a single sync-wait slot in this walrus build).
from concourse.vector_clock import ScopedClock as _ScopedClock, VectorClock as _VectorClock


def _split_drain_and_barrier(self, tick_clock, wait_clock):
    nc = self.nc
    vals = [int(x) for x in _re.findall(r"\d+", repr(tick_clock.global_clock))]
    procs = [(i, v) for i, v in enumerate(vals) if v > 0]
    for idx, val in procs:
        vc = _VectorClock([0] * len(vals))
        vc.require_at_least(idx, val)
        d = nc.sync.drain()
        wait_clock.add_sem_waits(d.ins, _ScopedClock({None: vc}))
    nc.all_engine_barrier()
    popped = nc._tile_sem_poison_stack.pop()
    assert popped is self._sem_poison
    nc.clear_and_free_semaphores(list(self.sems.allocated().values()))
    nc.all_engine_barrier()


tile.TileContext._drain_and_barrier = _split_drain_and_barrier
# ---------------------------------------------------------------------------

F32 = mybir.dt.float32
F32R = mybir.dt.float32r
AF = mybir.ActivationFunctionType
OP = mybir.AluOpType

B, C, D = 2, 2048, 1024
H, HD = 16, 64
SCALE = HD ** -0.5  # 0.125
P = 128
KT = D // P          # 8 contraction tiles for the projections
NCH = C // 512       # 4 i/n chunks of 512
HPC = 4              # heads per core
PAIRS = 2            # head pairs per core
GD = HPC * HD        # 256 features per core per tensor

_nc_cache = {}


def build_nc(reps=1):
    nc = bass.Bass("TRN2")

    hT_d = nc.dram_tensor("hT", [D, C], F32, kind="ExternalInput")
    wqkv_d = nc.dram_tensor("wqkv", [D, 3 * GD], F32, kind="ExternalInput")
    bqkv_d = nc.dram_tensor("bqkv", [3 * GD], F32, kind="ExternalInput")
    wo_d = nc.dram_tensor("wo", [GD, D], F32, kind="ExternalInput")
    masks_d = nc.dram_tensor("masks", [P, P], F32, kind="ExternalInput")
    out_d = nc.dram_tensor("partial", [C, D], F32, kind="ExternalOutput")
    DBG = os.environ.get("KERNEL_DEBUG") == "1"
    if DBG:
        dbg_q = nc.dram_tensor("dbg_q", [P, PAIRS, C], F32, kind="ExternalOutput")
        dbg_k = nc.dram_tensor("dbg_k", [P, PAIRS, C], F32, kind="ExternalOutput")
        dbg_v = nc.dram_tensor("dbg_v", [P, C // P, PAIRS, 130], F32, kind="ExternalOutput")
        dbg_o = nc.dram_tensor("dbg_o", [P, PAIRS, C], F32, kind="ExternalOutput")

    hT_v = hT_d[:, :].rearrange("(kt p) n -> p kt n", p=P)          # [128, 8, 2048]
    wqkv_v = wqkv_d[:, :].rearrange("(kt p) m -> p kt m", p=P)      # [128, 8, 768]
    wo_v = wo_d[:, :].rearrange("(p2 p) n -> p p2 n", p=P)          # [128, 2, 1024]


    with tile.TileContext(nc) as tc:
        import contextlib

        with contextlib.ExitStack() as ctx:
            persist = ctx.enter_context(tc.tile_pool(name="persist", bufs=1))
            dram = ctx.enter_context(tc.tile_pool(name="dram", bufs=3, space="DRAM"))

            # ---------------- persistent tiles ----------------
            wqkv_r = persist.tile([P, KT, 3 * GD], F32R)
            wo_r = persist.tile([P, PAIRS, D], F32R)
            # with narrowed diagonal blocks the only masked region is the
            # 128x128 triangle at the causal edge - one tile, plain f32
            masks_r = persist.tile([P, P], F32)
            qT_r = persist.tile([P, PAIRS, C], F32R)
            kT_r = persist.tile([P, PAIRS, C], F32R)
            # v_aug per pair: [v_even(64) | 1 | v_odd(64) | 1] = 130 cols
            v_aug = persist.tile([P, C // P, PAIRS, 130], F32R)
            outT_r = persist.tile([P, PAIRS, C], F32R)
            # one small tile: [bq(2) | bk(2) | bv(2) | ones(1)]
            smalls = persist.tile([P, 7], F32)
            bq_sb = smalls[:, 0:2]
            bk_sb = smalls[:, 2:4]
            bv_sb = smalls[:, 4:6]
            ones_f = smalls[:, 6:7]

            hrpool = ctx.enter_context(tc.tile_pool(name="hr", bufs=2))
            hr0_holder = [None]
            nc.vector.memset(ones_f, 1.0)
            nc.sync.dma_start(bq_sb, bqkv_d[0:GD].rearrange("(p2 d) -> d p2", d=P))
            nc.sync.dma_start(bk_sb, bqkv_d[GD:2 * GD].rearrange("(p2 d) -> d p2", d=P))
            nc.sync.dma_start(bv_sb, bqkv_d[2 * GD:3 * GD].rearrange("(p2 d) -> d p2", d=P))

            # ones columns of v_aug (positions 64, 129 within each pair block)
            v_ones_view = v_aug.rearrange("p j p2 (q e) -> p j (p2 q) e", e=65)
            nc.scalar.copy(
                v_ones_view[:, :, :, 64:65],
                ones_f[:, 0:1, None, None].to_broadcast((P, C // P, 2 * PAIRS, 1)),
            )

            # ---------------- load + round qkv weights, interleaved with the
            # first hidden chunk so the first matmul starts early ----
            hr0 = hrpool.tile([P, KT, 512], F32R, name="hr0")
            hr0_holder[0] = hr0
            # gpsimd DMAs cast f32 -> f32r in flight (bit-identical to a DVE
            # rounding copy, verified on HW) - no staging or convert passes.
            # kt=0 additionally rides the faster HWDGE queue, staged into a
            # not-yet-used region of qT_r and converted by the idle DVE, so
            # the first matmul's operands arrive before the SWDGE stream.
            ramp_fast = [True]
            for kt in range(1, KT):
                nc.gpsimd.dma_start(wqkv_r[:, kt, :], wqkv_v[:, kt, :])
                nc.gpsimd.dma_start(hr0[:, kt, :], hT_v[:, kt, 0:512])

            def load_masks_wo():
                nc.sync.dma_start(masks_r, masks_d[:, :])
                for p2 in range(PAIRS):
                    nc.gpsimd.dma_start(wo_r[:, p2, :], wo_v[:, p2, :])

            for rep in range(reps):
                with (
                    tc.tile_pool(name="psb", bufs=1, space="PSUM") as psb,
                    tc.tile_pool(name="pss", bufs=2, space="PSUM") as pss,
                    tc.tile_pool(name="psv2", bufs=1, space="PSUM") as psv2,
                    tc.tile_pool(name="epool", bufs=4) as epool,
                    tc.tile_pool(name="evpool", bufs=2) as evpool,
                    tc.tile_pool(name="rpool", bufs=2) as rpool,
                    tc.tile_pool(name="bcpool", bufs=3) as bcpool,
                    tc.tile_pool(name="osb", bufs=2) as osb,
                ):
                    v_view = v_aug.rearrange("p j p2 (q e) -> p j (p2 q) e", e=65)

                    if rep == 0 and ramp_fast[0]:
                        # kt=0 loads ride the faster HWDGE queue, staged in the
                        # (ramp-idle) projection-output slots and converted by
                        # the idle DVE - parallel to the SWDGE casting stream
                        ramp_fast[0] = False
                        st_w = osb.tile([P, D], F32, tag="o_s", name="st_w")
                        nc.sync.dma_start(st_w[:, 0:3 * GD], wqkv_v[:, 0, :])
                        nc.vector.tensor_copy(wqkv_r[:, 0, :], st_w[:, 0:3 * GD])
                        st_h = osb.tile([P, D], F32, tag="o_s", name="st_h")
                        nc.sync.dma_start(st_h[:, 0:512], hT_v[:, 0, 0:512])
                        nc.vector.tensor_copy(hr0[:, 0, :], st_h[:, 0:512])

                    def qkv_chunk(c4):
                        """QKV projection for seq chunk c4 (512 positions)."""
                        ns = slice(c4 * 512, (c4 + 1) * 512)
                        if rep == 0 and c4 == 0 and hr0_holder[0] is not None:
                            hr = hr0_holder[0]
                        else:
                            hr = hrpool.tile([P, KT, 512], F32R)
                            for kt in range(KT):
                                nc.gpsimd.dma_start(hr[:, kt, :], hT_v[:, kt, ns])

                        # q round (psb slot)
                        ps = psb.tile([P, PAIRS, 512], F32, tag="b")
                        for kt in range(KT):
                            fl = dict(start=(kt == 0), stop=(kt == KT - 1))
                            for p2 in range(PAIRS):
                                nc.tensor.matmul(
                                    ps[:, p2, :],
                                    lhsT=wqkv_r[:, kt, p2 * P:(p2 + 1) * P],
                                    rhs=hr[:, kt, :], **fl)
                        for p2 in range(PAIRS):
                            nc.vector.tensor_scalar(
                                qT_r[:, p2, ns], ps[:, p2, :],
                                bq_sb[:, p2:p2 + 1], None, OP.add)
                        # k round (borrowed scores slot - consecutive B rounds
                        # never share a psum slot, so no round-to-round WAR stall)
                        ps3 = pss.tile([P, 1024], F32, tag="s", name="ps3")
                        ps = ps3.rearrange("p (a b) -> p a b", a=PAIRS)
                        for kt in range(KT):
                            fl = dict(start=(kt == 0), stop=(kt == KT - 1))
                            for p2 in range(PAIRS):
                                nc.tensor.matmul(
                                    ps[:, p2, :],
                                    lhsT=wqkv_r[:, kt, GD + p2 * P:GD + (p2 + 1) * P],
                                    rhs=hr[:, kt, :], **fl)
                        for p2 in range(PAIRS):
                            nc.vector.tensor_scalar(
                                kT_r[:, p2, ns], ps[:, p2, :],
                                bk_sb[:, p2:p2 + 1], None, OP.add)
                        # v rounds (2 x 2 seq-subtiles; one bank per accum group)
                        for vr in range(2):
                            if vr == 0:
                                ps = psb.tile([P, PAIRS, 512], F32, tag="b")
                            else:
                                ps4 = pss.tile([P, 1024], F32, tag="s", name="ps4")
                                ps = ps4.rearrange("p (a b) -> p a b", a=PAIRS)
                            for kt in range(KT):
                                fl = dict(start=(kt == 0), stop=(kt == KT - 1))
                                for g in range(2):
                                    ms = 2 * vr + g
                                    nc.tensor.matmul(
                                        ps[:, g, 0:GD],
                                        lhsT=hr[:, kt, ms * P:(ms + 1) * P],
                                        rhs=wqkv_r[:, kt, 2 * GD:3 * GD], **fl)
                            for g in range(2):
                                jt = 4 * c4 + 2 * vr + g
                                nc.vector.tensor_copy(
                                    v_view[:, jt, :, 0:64],
                                    ps[:, g, 0:GD].rearrange("p (q d) -> p q d", d=64))

                    def attn_jt(p2, c4, jt, ps_pv, njt):
                        jsl = slice(jt * P, (jt + 1) * P)
                        k_off = jt - 4 * c4  # >=0 on diagonal blocks
                        i0 = max(0, k_off) * P  # first causal column in chunk
                        islw = slice(c4 * 512 + i0, (c4 + 1) * 512)
                        ps_s = pss.tile([P, 1024], F32, tag="s")
                        nc.tensor.matmul(
                            ps_s[:, i0:512],
                            lhsT=kT_r[0:64, p2, jsl], rhs=qT_r[0:64, p2, islw],
                            start=True, stop=True)
                        nc.tensor.matmul(
                            ps_s[:, 512 + i0:1024],
                            lhsT=kT_r[64:128, p2, jsl], rhs=qT_r[64:128, p2, islw],
                            start=True, stop=True)
                        E = epool.tile([P, 1024], F32R)
                        if i0 == 0:
                            nc.scalar.activation(E, ps_s, AF.Exp, scale=SCALE)
                        else:
                            nc.scalar.activation(
                                E[:, i0:512], ps_s[:, i0:512], AF.Exp, scale=SCALE)
                            nc.scalar.activation(
                                E[:, 512 + i0:1024], ps_s[:, 512 + i0:1024],
                                AF.Exp, scale=SCALE)
                        if k_off >= 0:
                            # only the leading 128 columns of the causal span
                            # are partially masked (triangular edge)
                            nc.vector.tensor_tensor(
                                E[:, i0:i0 + P], E[:, i0:i0 + P],
                                masks_r, OP.mult)
                            nc.vector.tensor_tensor(
                                E[:, 512 + i0:512 + i0 + P],
                                E[:, 512 + i0:512 + i0 + P],
                                masks_r, OP.mult)
                        fl = dict(start=(jt == 0), stop=(jt == njt - 1))
                        nc.tensor.matmul(
                            ps_pv[:, i0:512], lhsT=v_aug[:, jt, p2, 0:65],
                            rhs=E[:, i0:512], **fl)
                        nc.tensor.matmul(
                            ps_pv[:, 512 + i0:1024], lhsT=v_aug[:, jt, p2, 65:130],
                            rhs=E[:, 512 + i0:1024], **fl)

                    def attn_div(p2, c4, ps_pv, fast_tail=False):
                        """Evacuate PV psum, reciprocal via DMA lane-reshape, divide.

                        fast_tail: 2 DMA hops + a [64,1024] DVE reciprocal instead
                        of 4 hops + an 8-element one - shorter critical chain for
                        the final chunk where the latency is exposed."""
                        isl = slice(c4 * 512, (c4 + 1) * 512)
                        evac = evpool.tile([65, 1024], F32)
                        nc.vector.tensor_copy(evac, ps_pv)
                        scr1 = dram.tile([1024], F32, tag="scr1")
                        nc.sync.dma_start(scr1[None, :], evac[64:65, :])
                        bc = bcpool.tile([64, 1024], F32)
                        if fast_tail:
                            nc.sync.dma_start(bc, scr1[None, :].to_broadcast((64, 1024)))
                            nc.vector.reciprocal(bc, bc)
                        else:
                            rsh = rpool.tile([P, 8], F32)
                            nc.sync.dma_start(rsh, scr1.rearrange("(p e) -> p e", p=P))
                            nc.vector.reciprocal(rsh, rsh)
                            scr2 = dram.tile([1024], F32, tag="scr2")
                            nc.sync.dma_start(scr2.rearrange("(p e) -> p e", p=P), rsh)
                            nc.sync.dma_start(bc, scr2[None, :].to_broadcast((64, 1024)))
                        nc.vector.tensor_tensor(
                            outT_r[0:64, p2, isl], evac[0:64, 0:512],
                            bc[:, 0:512], OP.mult)
                        nc.vector.tensor_tensor(
                            outT_r[64:128, p2, isl], evac[0:64, 512:1024],
                            bc[:, 512:1024], OP.mult)
                        # + bv (softmax rows sum to 1 -> v bias passes through PV)
                        nc.vector.tensor_scalar(
                            outT_r[:, p2, isl], outT_r[:, p2, isl],
                            bv_sb[:, p2:p2 + 1], None, OP.add)

                    def attn_chunk(p2, c4):
                        ps_pv = psv2.tile([65, 1024], F32, tag="pv")
                        njt = 4 * c4 + 4
                        for jt in range(njt):
                            attn_jt(p2, c4, jt, ps_pv, njt)
                        attn_div(p2, c4, ps_pv, fast_tail=(c4 == NCH - 1))

                    def attn_chunk_both(c4):
                        """Both head pairs interleaved (used when the B-round psum
                        slot is free, i.e. the last chunk)."""
                        ps_pv0 = psv2.tile([65, 1024], F32, tag="pv")
                        pv1t = psb.tile([P, PAIRS, 512], F32, tag="b", name="pv1t")
                        ps_pv1 = pv1t.rearrange("p a b -> p (a b)")[0:65, :]
                        njt = 4 * c4 + 4
                        for jt in range(njt):
                            attn_jt(0, c4, jt, ps_pv0, njt)
                            attn_jt(1, c4, jt, ps_pv1, njt)
                        attn_div(0, c4, ps_pv0)
                        attn_div(1, c4, ps_pv1)

                    def proj_chunk(c4):
                        """Output projection rows of seq chunk c4 (4 r-tiles)."""
                        for rr in range(4):
                            r16 = 4 * c4 + rr
                            rsl = slice(r16 * P, (r16 + 1) * P)
                            ps_o3 = psb.tile([P, PAIRS, 512], F32, tag="b", name="ps_o3")
                            ps_o = ps_o3.rearrange("p a b -> p (a b)")
                            for n2 in range(2):
                                nsl = slice(n2 * 512, (n2 + 1) * 512)
                                for p2 in range(PAIRS):
                                    nc.tensor.matmul(
                                        ps_o[:, n2 * 512:(n2 + 1) * 512],
                                        lhsT=outT_r[:, p2, rsl],
                                        rhs=wo_r[:, p2, nsl],
                                        start=(p2 == 0), stop=(p2 == PAIRS - 1))
                            o_s = osb.tile([P, 1024], F32, tag="o_s")
                            if rr % 2 == 0:
                                nc.scalar.copy(o_s, ps_o)
                            else:
                                nc.vector.tensor_copy(o_s, ps_o)
                            nc.sync.dma_start(out_d[rsl, :], o_s)

                    # fused pipeline: QKV of chunk c+1 overlaps attention of chunk c,
                    # projection of chunk c-1 fills remaining PE slack
                    qkv_chunk(0)
                    if rep == 0:
                        load_masks_wo()
                    for c4 in range(NCH):
                        if c4 + 1 < NCH:
                            qkv_chunk(c4 + 1)
                        attn_chunk(0, c4)
                        attn_chunk(1, c4)
                        proj_chunk(c4)

                    if DBG:
                        nc.sync.dma_start(dbg_q[:, :, :], qT_r.bitcast(F32))
                        nc.sync.dma_start(dbg_k[:, :, :], kT_r.bitcast(F32))
                        nc.sync.dma_start(dbg_v[:, :, :, :], v_aug.bitcast(F32))
                        nc.sync.dma_start(dbg_o[:, :, :], outT_r.bitcast(F32))


    import bass_rust as _br
    _br.move_matmul_waits_to_ldweights(nc.m)
    _br.generate_event_semaphores(nc)
    nc.finalize()
    return nc


def _make_masks():
    j = np.arange(P)[:, None]
    i = np.arange(P)[None, :]
    return (i >= j).astype(np.float32)


def _prep_inputs(hidden_states, Wqkv, bqkv, Wo):
    masks = _make_masks()
    in_maps = []
    for c in range(8):
        b, g = c // 4, c % 4
        hT = np.ascontiguousarray(hidden_states[b].T)  # [1024, 2048]
        qs = slice(g * GD, (g + 1) * GD)
        wq = Wqkv[:, qs]
        wk = Wqkv[:, D + g * GD:D + (g + 1) * GD]
        wv = Wqkv[:, 2 * D + g * GD:2 * D + (g + 1) * GD]
        wqkv_c = np.ascontiguousarray(np.concatenate([wq, wk, wv], axis=1))
        bqkv_c = np.ascontiguousarray(np.concatenate(
            [bqkv[qs], bqkv[D + g * GD:D + (g + 1) * GD],
             bqkv[2 * D + g * GD:2 * D + (g + 1) * GD]]))
        wo_c = np.ascontiguousarray(Wo[g * GD:(g + 1) * GD, :])
        in_maps.append({
            "hT": hT, "wqkv": wqkv_c, "bqkv": bqkv_c, "wo": wo_c, "masks": masks,
        })
    return in_maps


_last_results = None


def kernel(hidden_states, attention_mask, Wqkv, bqkv, Wo, bo):
    """Full-input, full-output causal self-attention on 8 NeuronCores."""
    global _last_results
    from concourse.bass_utils import run_bass_kernel_spmd

    hidden_states = np.asarray(hidden_states, dtype=np.float32)
    Wqkv = np.asarray(Wqkv, dtype=np.float32)
    bqkv = np.asarray(bqkv, dtype=np.float32)
    Wo = np.asarray(Wo, dtype=np.float32)
    bo = np.asarray(bo, dtype=np.float32)

    if "nc" not in _nc_cache:
        _nc_cache["nc"] = build_nc()
    nc = _nc_cache["nc"]

    in_maps = _prep_inputs(hidden_states, Wqkv, bqkv, Wo)
    res = run_bass_kernel_spmd(nc, in_maps, core_ids=list(range(8)))
    _last_results = res

    parts = [r["partial"] for r in res.results]
    out = np.empty((B, C, D), dtype=np.float32)
    for b in range(B):
        acc = parts[4 * b].astype(np.float64)
        for g in range(1, 4):
            acc = acc + parts[4 * b + g]
        out[b] = (acc + bo.astype(np.float64)).astype(np.float32)
    return out



# revision 6
# speedup vs baseline: 1.2308x; 1.2308x over previous
"""Causal self-attention Trainium2 kernel (8 NeuronCores, batch x head-group sharded).

Problem: hidden [2, 2048, 1024], 16 heads x 64 dim, causal softmax attention,
QKV projection + output projection, all fp32 in/out.

Sharding: core c handles batch b = c//4 and head group g = c%4 (4 heads).
Each core computes qkv projections for its 4 heads, causal attention, and a
partial output projection (row-shard of Wo). Host sums the 4 partials per
batch and adds bo.

Numerics / engine strategy:
  - QKV projection runs as fp8e4m3 DoubleRow matmuls on host-pre-split
    operands (x ~= hi + lo, both e4m3; products hi*hi + hi*lo + lo*hi).
    Error ~2^-7 relative, at 2.67x the fp32r matmul rate.
  - q/k/v/E and the attention output are bf16; scores accumulate in fp32
    PSUM; softmax denominators ride the PV matmul as a 65th "ones" column.
  - PV uses the q-stationary orientation: out[q, f] = sum_j E[j,q] v[j,f],
    so each 128-key block costs 65 PE rows instead of ~1024, and the
    softmax denominator lands per-partition (reciprocal is one tiny DVE op,
    broadcast along free). A bf16 PE transpose (identity moving operand)
    restores the feature-major layout for the output projection.
  - The emission schedule software-pipelines: scores(jt+1) and QKV/proj
    "filler" matmuls execute while ACT runs exp(jt); PV(jt) lands after.
"""
import collections
import math
import os
import re as _re

import numpy as np
import ml_dtypes

import concourse.bass as bass
import concourse.mybir as mybir
import concourse.tile as tile

# ---------------------------------------------------------------------------
# The final TileContext drain carries one wait per proc (the Drain instruction
# has a single sync-wait slot in this walrus build).
from concourse.vector_clock import ScopedClock as _ScopedClock, VectorClock as _VectorClock


def _split_drain_and_barrier(self, tick_clock, wait_clock):
    nc = self.nc
    vals = [int(x) for x in _re.findall(r"\d+", repr(tick_clock.global_clock))]
    procs = [(i, v) for i, v in enumerate(vals) if v > 0]
    for idx, val in procs:
        vc = _VectorClock([0] * len(vals))
        vc.require_at_least(idx, val)
        d = nc.sync.drain()
        wait_clock.add_sem_waits(d.ins, _ScopedClock({None: vc}))
    nc.all_engine_barrier()
    popped = nc._tile_sem_poison_stack.pop()
    assert popped is self._sem_poison
    nc.clear_and_free_semaphores(list(self.sems.allocated().values()))
    nc.all_engine_barrier()


tile.TileContext._drain_and_barrier = _split_drain_and_barrier
# ---------------------------------------------------------------------------

F32 = mybir.dt.float32
BF16 = mybir.dt.bfloat16
F8 = mybir.dt.float8e4
AF = mybir.ActivationFunctionType
OP = mybir.AluOpType
DR = mybir.MatmulPerfMode.DoubleRow

B, C, D = 2, 2048, 1024
H, HD = 16, 64
SCALE = HD ** -0.5  # 0.125
WSCALE = 64.0        # host scales Wqkv by this before fp8 split (fp8 subnormal
                     # underflow otherwise: sigma_W ~ 0.03); q/k carry x64 each
                     # -> exp scale absorbs 1/WSCALE^2; v's x64 folds into Wo
P = 128
KT = D // P          # 8 contraction tiles of 128
NCH = C // 512       # 4 seq chunks of 512
HPC = 4              # heads per core
PAIRS = 2            # head pairs per core
GD = HPC * HD        # 256 features per core per tensor

_nc_cache = {}


def build_nc():
    nc = bass.Bass("TRN2")

    h8_d = nc.dram_tensor("h8", [2, D, C], F8, kind="ExternalInput")      # [hl, d, n]
    w8_d = nc.dram_tensor("w8", [2, D, 3 * GD], F8, kind="ExternalInput") # [hl, d, m]
    wo_d = nc.dram_tensor("wo", [GD, D], BF16, kind="ExternalInput")
    bqkv_d = nc.dram_tensor("bqkv", [3 * GD], F32, kind="ExternalInput")
    masks_d = nc.dram_tensor("masks", [P, P], BF16, kind="ExternalInput")
    ident_d = nc.dram_tensor("ident", [P, P], BF16, kind="ExternalInput")
    out_d = nc.dram_tensor("partial", [C, D], F32, kind="ExternalOutput")
    DBG = os.environ.get("KERNEL_DEBUG") == "1"
    if DBG:
        dbg_q = nc.dram_tensor("dbg_q", [P, PAIRS, C], BF16, kind="ExternalOutput")
        dbg_k = nc.dram_tensor("dbg_k", [P, PAIRS, C], BF16, kind="ExternalOutput")
        dbg_v = nc.dram_tensor("dbg_v", [P, C // P, HPC, 65], BF16, kind="ExternalOutput")
        dbg_o = nc.dram_tensor("dbg_o", [P, PAIRS, C], BF16, kind="ExternalOutput")

    h8_v = h8_d[:, :, :].rearrange("hl (kt p) n -> p hl kt n", p=P)   # [128, 2, 8, 2048]
    w8_v = w8_d[:, :, :].rearrange("hl (kt p) m -> p hl kt m", p=P)   # [128, 2, 8, 768]
    wo_v = wo_d[:, :].rearrange("(p2 p) n -> p p2 n", p=P)            # [128, 2, 1024]

    with tile.TileContext(nc) as tc:
        import contextlib

        with contextlib.ExitStack() as ctx:
            persist = ctx.enter_context(tc.tile_pool(name="persist", bufs=1))
            h8p = ctx.enter_context(tc.tile_pool(name="h8p", bufs=3))
            epool = ctx.enter_context(tc.tile_pool(name="epool", bufs=3))
            apool = ctx.enter_context(tc.tile_pool(name="apool", bufs=3))
            rpool = ctx.enter_context(tc.tile_pool(name="rpool", bufs=3))
            osb = ctx.enter_context(tc.tile_pool(name="osb", bufs=2))
            pqkv = ctx.enter_context(tc.tile_pool(name="pqkv", bufs=2, space="PSUM"))
            pss = ctx.enter_context(tc.tile_pool(name="pss", bufs=2, space="PSUM"))
            ppv = ctx.enter_context(tc.tile_pool(name="ppv", bufs=2, space="PSUM"))

            # ---------------- persistent tiles ----------------
            w8_r = persist.tile([P, 2, KT, 3 * GD], F8)        # 12KB/part
            wo_r = persist.tile([P, PAIRS, D], BF16)           # 4KB
            masks_r = persist.tile([P, P], BF16)
            ident_r = persist.tile([P, P], BF16)
            qT_r = persist.tile([P, PAIRS, C], BF16)           # 8KB
            kT_r = persist.tile([P, PAIRS, C], BF16)           # 8KB
            # v per key-block jt, head hh (= 2*p2+h): [v(64) | 1]
            v_r = persist.tile([P, C // P, HPC, 65], BF16)     # 8.3KB
            outT_r = persist.tile([P, PAIRS, C], BF16)         # 8KB
            smalls = persist.tile([P, 6], F32)
            bq_sb = smalls[:, 0:2]
            bk_sb = smalls[:, 2:4]
            bv_sb = smalls[:, 4:6]

            # ---------------- one-time loads ----------------
            nc.sync.dma_start(masks_r, masks_d[:, :])
            nc.sync.dma_start(ident_r, ident_d[:, :])
            nc.sync.dma_start(bq_sb, bqkv_d[0:GD].rearrange("(p2 d) -> d p2", d=P))
            nc.sync.dma_start(bk_sb, bqkv_d[GD:2 * GD].rearrange("(p2 d) -> d p2", d=P))
            nc.sync.dma_start(bv_sb, bqkv_d[2 * GD:3 * GD].rearrange("(p2 d) -> d p2", d=P))
            for hl in range(2):
                nc.gpsimd.dma_start(w8_r[:, hl, :, :], w8_v[:, hl, :, :])
            nc.sync.dma_start(wo_r[:, :, :], wo_v[:, :, :])
            nc.vector.memset(v_r[:, :, :, 64:65], 1.0)

            # h chunk tiles, prefetched 2 chunks ahead
            h8t = {}

            def prefetch_h(c4):
                t = h8p.tile([P, 2, KT, 512], F8, name=f"h8_{c4}", tag="h8")
                ns = slice(c4 * 512, (c4 + 1) * 512)
                for hl in range(2):
                    nc.gpsimd.dma_start(t[:, hl, :, :], h8_v[:, hl, :, ns])
                h8t[c4] = t

            prefetch_h(0)
            prefetch_h(1)

            # ---------------- work-closure machinery ----------------
            filler = collections.deque()

            def drain(n):
                for _ in range(min(n, len(filler))):
                    filler.popleft()()

            # fp8 DoubleRow split-matmul term list: (w_hl, h_hl)
            TERMS = ((0, 0), (0, 1), (1, 0))

            def qkv_closures(c4):
                """Emit-closures for the qkv projection of chunk c4."""
                ns0 = c4 * 512
                ht = h8t[c4]
                out = []

                def qk_round(tgt, p2):
                    # tgt: 0=q, 1=k ; psum [128, 2, 256] = two 256-seq subtiles
                    fsl = slice(tgt * GD + p2 * P, tgt * GD + (p2 + 1) * P)
                    ps = pqkv.tile([P, 2, 256], F32, name=f"qk{c4}_{tgt}{p2}", tag="pq")

                    def mk(sub):
                        def go():
                            nsub = slice(sub * 256, (sub + 1) * 256)
                            first = sub == 0
                            n = 0
                            for ktp in range(KT // 2):
                                ksl = slice(2 * ktp, 2 * ktp + 2)
                                for (wl, hl) in TERMS:
                                    nc.tensor.matmul(
                                        ps[:, sub, :],
                                        lhsT=w8_r[:, wl, ksl, fsl],
                                        rhs=ht[:, hl, ksl, nsub],
                                        start=(first and n == 0),
                                        stop=(not first and n == 11),
                                        perf_mode=DR, skip_group_check=True)
                                    n += 1
                        return go
                    def evac():
                        bias = (bq_sb, bk_sb)[tgt]
                        dst = (qT_r, kT_r)[tgt]
                        nc.vector.tensor_scalar(
                            dst[:, p2, ns0:ns0 + 512],
                            ps.rearrange("p a b -> p (a b)"),
                            bias[:, p2:p2 + 1], None, OP.add)
                    return [mk(0), mk(1), evac]

                def v_round(vr):
                    # two key-blocks (ms = 2*vr, 2*vr+1); psum [128, 2, 256]
                    ps = pqkv.tile([P, 2, 256], F32, name=f"v{c4}_{vr}", tag="pq")

                    def mk(g):
                        def go():
                            ms = 2 * vr + g
                            msl = slice(ms * P, (ms + 1) * P)
                            n = 0
                            for ktp in range(KT // 2):
                                ksl = slice(2 * ktp, 2 * ktp + 2)
                                for (wl, hl) in TERMS:
                                    nc.tensor.matmul(
                                        ps[:, g, :],
                                        lhsT=ht[:, hl, ksl, msl],
                                        rhs=w8_r[:, wl, ksl, 2 * GD:3 * GD],
                                        start=(g == 0 and n == 0),
                                        stop=(g == 1 and n == 11),
                                        perf_mode=DR, skip_group_check=True)
                                    n += 1
                        return go
                    def evac():
                        jt = 4 * c4 + 2 * vr
                        nc.vector.tensor_copy(
                            v_r[:, jt:jt + 2, :, 0:64],
                            ps.rearrange("p g (hh fd) -> p g hh fd", fd=64))
                    return [mk(0), mk(1), evac]

                for p2 in range(PAIRS):
                    out.append(qk_round(0, p2))
                for p2 in range(PAIRS):
                    out.append(qk_round(1, p2))
                for vr in range(2):
                    out.append(v_round(vr))
                res = []
                for r in out:
                    a, b, ev = r
                    res.append(a)
                    # evac rides with the second half-closure
                    res.append(lambda b=b, ev=ev: (b(), ev()))
                return res

            def proj_closures(c4):
                """Output projection of chunk c4 (reads outT_r, writes out_d)."""
                res = []
                for rr in range(4):
                    r16 = 4 * c4 + rr
                    rsl = slice(r16 * P, (r16 + 1) * P)
                    ot = [None]

                    def half(n2, rr=rr, r16=r16, rsl=rsl, ot=ot):
                        def go():
                            nsl = slice(n2 * 512, (n2 + 1) * 512)
                            ps = pqkv.tile([P, 512], F32, name=f"pr{r16}_{n2}", tag="pq")
                            for p2 in range(PAIRS):
                                nc.tensor.matmul(
                                    ps, lhsT=outT_r[:, p2, rsl],
                                    rhs=wo_r[:, p2, nsl],
                                    start=(p2 == 0), stop=(p2 == PAIRS - 1))
                            if n2 == 0:
                                ot[0] = osb.tile([P, D], F32, name=f"os{r16}", tag="os")
                            nc.vector.tensor_copy(ot[0][:, nsl], ps)
                            if n2 == 1:
                                nc.sync.dma_start(out_d[rsl, :], ot[0])
                        return go
                    res.append(half(0))
                    res.append(half(1))
                return res

            # ---------------- attention ----------------
            def attn_p2(p2, c4):
                njt = 4 * c4 + 4
                isl0 = c4 * 512
                pv = [ppv.tile([P, 2, 2, 65], F32, name=f"pv{p2}{c4}a", tag="pv"),
                      ppv.tile([P, 2, 2, 65], F32, name=f"pv{p2}{c4}b", tag="pv")]
                started = [False, False]
                tsp = [None, None]

                def scores_step(jt):
                    i0 = max(0, jt - 4 * c4) * P
                    jsl = slice(jt * P, (jt + 1) * P)
                    isw = slice(isl0 + i0, isl0 + 512)
                    ps_s = pss.tile([P, 1024], F32, name=f"s{p2}{c4}_{jt % 2}", tag="s")
                    for h in range(2):
                        nc.tensor.matmul(
                            ps_s[:, 512 * h + i0:512 * h + 512],
                            lhsT=kT_r[64 * h:64 * h + 64, p2, jsl],
                            rhs=qT_r[64 * h:64 * h + 64, p2, isw],
                            start=True, stop=True)
                    E = epool.tile([P, 1024], BF16, name=f"E{jt % 3}", tag="E")
                    if i0 == 0:
                        nc.scalar.activation(E, ps_s, AF.Exp, scale=SCALE / (WSCALE * WSCALE))
                    else:
                        ev = E.rearrange("p (h n) -> p h n", h=2)[:, :, i0:512]
                        sv = ps_s.rearrange("p (h n) -> p h n", h=2)[:, :, i0:512]
                        nc.scalar.activation(ev, sv, AF.Exp, scale=SCALE / (WSCALE * WSCALE))
                    if jt >= 4 * c4:
                        mv = E.rearrange("p (h n) -> p h n", h=2)[:, :, i0:i0 + P]
                        nc.gpsimd.tensor_tensor(
                            mv, mv,
                            masks_r[:, None, :].to_broadcast((P, 2, P)), OP.mult)
                    return E

                def pv_step(jt, E):
                    def go():
                        qs0 = max(0, jt - 4 * c4)
                        for qs in range(qs0, 4):
                            bk = qs // 2
                            for h in range(2):
                                st = not started[bk]
                                started[bk] = True
                                nc.tensor.matmul(
                                    pv[bk][:, qs % 2, h, :],
                                    lhsT=E[:, 512 * h + 128 * qs:512 * h + 128 * qs + 128],
                                    rhs=v_r[:, jt, 2 * p2 + h, :],
                                    start=st,
                                    stop=(jt == 4 * c4 + (2 * bk + 1) and qs == 2 * bk + 1 and h == 1),
                                    perf_mode=None, skip_group_check=True)
                    return go

                def end_bank(bk):
                    def go():
                        rec = rpool.tile([P, 2, 2, 1], F32, name=f"r{p2}{bk}", tag="r")
                        nc.vector.reciprocal(rec, pv[bk][:, :, :, 64:65])
                        at = apool.tile([P, 2, 2, 64], BF16, name=f"a{p2}{bk}", tag="a")
                        nc.vector.tensor_tensor(
                            at, pv[bk][:, :, :, 0:64],
                            rec.to_broadcast((P, 2, 2, 64)), OP.mult)
                        return at
                    return go

                def transp_bank(bk, at):
                    def go():
                        t = ppv.tile([P, 2, P], BF16, name=f"t{p2}{bk}", tag="pv")
                        tsp[bk] = t
                        for q in range(2):
                            nc.tensor.transpose(
                                t[:, q, :], at[:, q, :, :].rearrange("p h f -> p (h f)"),
                                ident_r)
                    return go

                def evac2_bank(bk):
                    def go():
                        qsl = slice(isl0 + 256 * bk, isl0 + 256 * (bk + 1))
                        nc.vector.tensor_scalar(
                            outT_r[:, p2, qsl],
                            tsp[bk].rearrange("p a b -> p (a b)"),
                            bv_sb[:, p2:p2 + 1], None, OP.add)
                    return go

                pend_pv = None
                post = collections.deque()  # delayed endgame closures
                for jt in range(njt):
                    E = scores_step(jt)
                    drain(2)
                    if post:
                        post.popleft()()
                    if pend_pv is not None:
                        pend_pv()
                        if jt == 4 * c4 + 2:
                            # bank0 stopped at jt-1's pv (just emitted)
                            atA = end_bank(0)()
                            post.append(transp_bank(0, atA))
                            post.append(evac2_bank(0))
                    pend_pv = pv_step(jt, E)
                pend_pv()
                atB = end_bank(1)()
                while post:
                    drain(1)
                    post.popleft()()
                drain(2)
                transp_bank(1, atB)()
                drain(1)
                evac2_bank(1)()

            # ---------------- main pipeline ----------------
            for cl in qkv_closures(0):
                cl()
            for c4 in range(NCH):
                if c4 + 1 < NCH:
                    filler.extend(qkv_closures(c4 + 1))
                if c4 + 2 < NCH:
                    prefetch_h(c4 + 2)
                attn_p2(0, c4)
                attn_p2(1, c4)
                filler.extend(proj_closures(c4))
            while filler:
                filler.popleft()()

            if DBG:
                nc.sync.dma_start(dbg_q[:, :, :], qT_r)
                nc.sync.dma_start(dbg_k[:, :, :], kT_r)
                nc.sync.dma_start(dbg_v[:, :, :, :], v_r)
                nc.sync.dma_start(dbg_o[:, :, :], outT_r)

    import bass_rust as _br
    _br.move_matmul_waits_to_ldweights(nc.m)
    _br.generate_event_semaphores(nc)
    nc.finalize()
    return nc


E4 = ml_dtypes.float8_e4m3
BF = ml_dtypes.bfloat16


def _split8(x):
    hi = x.astype(E4)
    lo = (x - hi.astype(np.float32)).astype(E4)
    return np.stack([hi, lo])


def _prep_inputs(hidden_states, Wqkv, bqkv, Wo):
    j = np.arange(P)[:, None]
    i = np.arange(P)[None, :]
    masks = (i >= j).astype(BF)
    ident = np.eye(P, dtype=np.float32).astype(BF)
    in_maps = []
    h8_cache = {}
    for c in range(8):
        b, g = c // 4, c % 4
        if b not in h8_cache:
            h8_cache[b] = _split8(np.ascontiguousarray(hidden_states[b].T))
        qs = slice(g * GD, (g + 1) * GD)
        wq = Wqkv[:, qs]
        wk = Wqkv[:, D + g * GD:D + (g + 1) * GD]
        wv = Wqkv[:, 2 * D + g * GD:2 * D + (g + 1) * GD]
        w_cat = np.ascontiguousarray(np.concatenate([wq, wk, wv], axis=1)) * np.float32(WSCALE)
        bqkv_c = np.ascontiguousarray(np.concatenate(
            [bqkv[qs], bqkv[D + g * GD:D + (g + 1) * GD],
             bqkv[2 * D + g * GD:2 * D + (g + 1) * GD]])).astype(np.float32) * np.float32(WSCALE)
        wo_c = np.ascontiguousarray(Wo[g * GD:(g + 1) * GD, :] / WSCALE).astype(BF)
        in_maps.append({
            "h8": h8_cache[b], "w8": _split8(w_cat), "wo": wo_c,
            "bqkv": bqkv_c, "masks": masks, "ident": ident,
        })
    return in_maps


_last_results = None


def kernel(hidden_states, attention_mask, Wqkv, bqkv, Wo, bo):
    """Full-input, full-output causal self-attention on 8 NeuronCores."""
    global _last_results
    from concourse.bass_utils import run_bass_kernel_spmd

    hidden_states = np.asarray(hidden_states, dtype=np.float32)
    Wqkv = np.asarray(Wqkv, dtype=np.float32)
    bqkv = np.asarray(bqkv, dtype=np.float32)
    Wo = np.asarray(Wo, dtype=np.float32)
    bo = np.asarray(bo, dtype=np.float32)

    if "nc" not in _nc_cache:
        _nc_cache["nc"] = build_nc()
    nc = _nc_cache["nc"]

    in_maps = _prep_inputs(hidden_states, Wqkv, bqkv, Wo)
    res = run_bass_kernel_spmd(nc, in_maps, core_ids=list(range(8)))
    _last_results = res

    parts = [r["partial"] for r in res.results]
    out = np.empty((B, C, D), dtype=np.float32)
    for b in range(B):
        acc = parts[4 * b].astype(np.float64)
        for g in range(1, 4):
            acc = acc + parts[4 * b + g]
        out[b] = (acc + bo.astype(np.float64)).astype(np.float32)
    return out


# revision 8
# speedup vs baseline: 1.3693x; 1.1125x over previous
"""Causal self-attention Trainium2 kernel (8 NeuronCores, batch x head-group sharded).

Problem: hidden [2, 2048, 1024], 16 heads x 64 dim, causal softmax attention,
QKV projection + output projection, all fp32 in/out.

Sharding: core c handles batch b = c//4 and head group g = c%4 (4 heads).
Each core computes qkv projections for its 4 heads, causal attention, and a
partial output projection (row-shard of Wo). Host sums the 4 partials per
batch and adds bo.

Numerics / engine strategy:
  - QKV projection runs as fp8e4m3 DoubleRow matmuls on host-pre-split
    operands (x ~= hi + lo, both e4m3; products hi*hi + hi*lo + lo*hi).
    Error ~2^-7 relative, at 2.67x the fp32r matmul rate.
  - q/k/v/E and the attention output are bf16; scores accumulate in fp32
    PSUM; softmax denominators ride the PV matmul as a 65th "ones" column.
  - PV uses the q-stationary orientation: out[q, f] = sum_j E[j,q] v[j,f],
    so each 128-key block costs 65 PE rows instead of ~1024, and the
    softmax denominator lands per-partition (reciprocal is one tiny DVE op,
    broadcast along free). A bf16 PE transpose (identity moving operand)
    restores the feature-major layout for the output projection.
  - The emission schedule software-pipelines: scores(jt+1) and QKV/proj
    "filler" matmuls execute while ACT runs exp(jt); PV(jt) lands after.
"""
import collections
import math
import os
import re as _re

import numpy as np
import ml_dtypes

import concourse.bass as bass
import concourse.mybir as mybir
import concourse.tile as tile

# ---------------------------------------------------------------------------
# The final TileContext drain carries one wait per proc (the Drain instruction
# has a single sync-wait slot in this walrus build).
from concourse.vector_clock import ScopedClock as _ScopedClock, VectorClock as _VectorClock


def _split_drain_and_barrier(self, tick_clock, wait_clock):
    nc = self.nc
    vals = [int(x) for x in _re.findall(r"\d+", repr(tick_clock.global_clock))]
    procs = [(i, v) for i, v in enumerate(vals) if v > 0]
    for idx, val in procs:
        vc = _VectorClock([0] * len(vals))
        vc.require_at_least(idx, val)
        d = nc.sync.drain()
        wait_clock.add_sem_waits(d.ins, _ScopedClock({None: vc}))
    nc.all_engine_barrier()
    popped = nc._tile_sem_poison_stack.pop()
    assert popped is self._sem_poison
    nc.clear_and_free_semaphores(list(self.sems.allocated().values()))
    nc.all_engine_barrier()


tile.TileContext._drain_and_barrier = _split_drain_and_barrier
# ---------------------------------------------------------------------------

F32 = mybir.dt.float32
BF16 = mybir.dt.bfloat16
F8 = mybir.dt.float8e4
AF = mybir.ActivationFunctionType
OP = mybir.AluOpType
DR = mybir.MatmulPerfMode.DoubleRow

B, C, D = 2, 2048, 1024
H, HD = 16, 64
SCALE = HD ** -0.5  # 0.125
WSCALE = 64.0        # host scales Wqkv by this before fp8 split (fp8 subnormal
                     # underflow otherwise: sigma_W ~ 0.03); q/k carry x64 each
                     # -> exp scale absorbs 1/WSCALE^2; v's x64 folds into Wo
P = 128
KT = D // P          # 8 contraction tiles of 128
NCH = C // 512       # 4 seq chunks of 512
HPC = 4              # heads per core
PAIRS = 2            # head pairs per core
GD = HPC * HD        # 256 features per core per tensor

_nc_cache = {}


def build_nc():
    nc = bass.Bass("TRN2")

    h8_d = nc.dram_tensor("h8", [2, D, C], F8, kind="ExternalInput")      # [hl, d, n]
    w8_d = nc.dram_tensor("w8", [2, D, 3 * GD], F8, kind="ExternalInput") # [hl, d, m]
    wo_d = nc.dram_tensor("wo", [GD, D], BF16, kind="ExternalInput")
    bqkv_d = nc.dram_tensor("bqkv", [3 * GD], F32, kind="ExternalInput")
    masks_d = nc.dram_tensor("masks", [P, P], BF16, kind="ExternalInput")
    ident_d = nc.dram_tensor("ident", [P, P], BF16, kind="ExternalInput")
    out_d = nc.dram_tensor("partial", [C, D], BF16, kind="ExternalOutput")
    DBG = os.environ.get("KERNEL_DEBUG") == "1"
    if DBG:
        dbg_q = nc.dram_tensor("dbg_q", [P, PAIRS, C], BF16, kind="ExternalOutput")
        dbg_k = nc.dram_tensor("dbg_k", [P, PAIRS, C], BF16, kind="ExternalOutput")
        dbg_v = nc.dram_tensor("dbg_v", [P, C // P, HPC, 65], BF16, kind="ExternalOutput")
        dbg_o = nc.dram_tensor("dbg_o", [P, PAIRS, C], BF16, kind="ExternalOutput")

    h8_v = h8_d[:, :, :].rearrange("hl (kt p) n -> p hl kt n", p=P)   # [128, 2, 8, 2048]
    w8_v = w8_d[:, :, :].rearrange("hl (kt p) m -> p hl kt m", p=P)   # [128, 2, 8, 768]
    wo_v = wo_d[:, :].rearrange("(p2 p) n -> p p2 n", p=P)            # [128, 2, 1024]

    with tile.TileContext(nc) as tc:
        import contextlib

        with contextlib.ExitStack() as ctx:
            persist = ctx.enter_context(tc.tile_pool(name="persist", bufs=1))
            h8p = ctx.enter_context(tc.tile_pool(name="h8p", bufs=3))
            epool = ctx.enter_context(tc.tile_pool(name="epool", bufs=3))
            apool = ctx.enter_context(tc.tile_pool(name="apool", bufs=3))
            rpool = ctx.enter_context(tc.tile_pool(name="rpool", bufs=3))
            osb = ctx.enter_context(tc.tile_pool(name="osb", bufs=2))
            pqkv = ctx.enter_context(tc.tile_pool(name="pqkv", bufs=2, space="PSUM"))
            pss = ctx.enter_context(tc.tile_pool(name="pss", bufs=2, space="PSUM"))
            ppv = ctx.enter_context(tc.tile_pool(name="ppv", bufs=2, space="PSUM"))

            # ---------------- persistent tiles ----------------
            w8_r = persist.tile([P, 2, KT, 3 * GD], F8)        # 12KB/part
            wo_r = persist.tile([P, PAIRS, D], BF16)           # 4KB
            masks_r = persist.tile([P, P], BF16)
            ident_r = persist.tile([P, P], BF16)
            qT_r = persist.tile([P, PAIRS, C], BF16)           # 8KB
            kT_r = persist.tile([P, PAIRS, C], BF16)           # 8KB
            # v per key-block jt, head hh (= 2*p2+h): [v(64) | 1]
            v_r = persist.tile([P, C // P, HPC, 65], BF16)     # 8.3KB
            outT_r = persist.tile([P, PAIRS, C], BF16)         # 8KB
            smalls = persist.tile([P, 6], F32)
            bq_sb = smalls[:, 0:2]
            bk_sb = smalls[:, 2:4]
            bv_sb = smalls[:, 4:6]

            # ---------------- one-time loads ----------------
            # Critical-path data first: the first hi*hi matmul needs w8-hi
            # (Pool SWDGE) and h8(0)-hi (SP HWDGE, queued ahead of the small
            # config loads). Everything else streams behind.
            h8t = {}

            def prefetch_h(c4, eng=None):
                t = h8p.tile([P, 2, KT, 512], F8, name=f"h8_{c4}", tag="h8")
                ns = slice(c4 * 512, (c4 + 1) * 512)
                for hl in range(2):
                    e = eng[hl] if eng else nc.gpsimd
                    e.dma_start(t[:, hl, :, :], h8_v[:, hl, :, ns])
                h8t[c4] = t

            nc.gpsimd.dma_start(w8_r[:, 0, :, :], w8_v[:, 0, :, :])
            prefetch_h(0, eng=(nc.sync, nc.sync))
            nc.sync.dma_start(bq_sb, bqkv_d[0:GD].rearrange("(p2 d) -> d p2", d=P))
            nc.sync.dma_start(bk_sb, bqkv_d[GD:2 * GD].rearrange("(p2 d) -> d p2", d=P))
            nc.scalar.dma_start(w8_r[:, 1, :, :], w8_v[:, 1, :, :])
            nc.sync.dma_start(bv_sb, bqkv_d[2 * GD:3 * GD].rearrange("(p2 d) -> d p2", d=P))
            nc.sync.dma_start(masks_r, masks_d[:, :])
            nc.sync.dma_start(wo_r[:, :, :], wo_v[:, :, :])
            nc.sync.dma_start(ident_r, ident_d[:, :])
            nc.vector.memset(v_r[:, :, :, 64:65], 1.0)
            prefetch_h(1, eng=(nc.scalar, nc.gpsimd))

            # ---------------- work-closure machinery ----------------
            filler = collections.deque()

            def drain(n):
                for _ in range(min(n, len(filler))):
                    filler.popleft()()

            pace = {"total": 0, "step": 0, "items": 0, "done": 0}

            def pace_chunk(total_steps):
                pace.update(total=total_steps, step=0, items=len(filler), done=0)

            def pstep():
                pace["step"] += 1
                tgt = -(-pace["items"] * pace["step"] // pace["total"])  # ceil
                k = min(tgt - pace["done"], len(filler))
                if k > 0:
                    pace["done"] += k
                    drain(k)

            # fp8 DoubleRow split-matmul term list: (w_hl, h_hl)
            TERMS = ((0, 0), (0, 1), (1, 0))

            def qkv_closures(c4):
                """Emit-closures for the qkv projection of chunk c4."""
                ns0 = c4 * 512
                ht = h8t[c4]
                out = []

                def qk_round(tgt, p2):
                    # tgt: 0=q, 1=k ; psum [128, 2, 256] = two 256-seq subtiles
                    fsl = slice(tgt * GD + p2 * P, tgt * GD + (p2 + 1) * P)
                    ps = pqkv.tile([P, 2, 256], F32, name=f"qk{c4}_{tgt}{p2}", tag="pq")

                    def mk(sub):
                        def go():
                            nsub = slice(sub * 256, (sub + 1) * 256)
                            first = sub == 0
                            n = 0
                            for (wl, hl) in TERMS:
                                for ktp in range(KT // 2):
                                    ksl = slice(2 * ktp, 2 * ktp + 2)
                                    nc.tensor.matmul(
                                        ps[:, sub, :],
                                        lhsT=w8_r[:, wl, ksl, fsl],
                                        rhs=ht[:, hl, ksl, nsub],
                                        start=(first and n == 0),
                                        stop=(not first and n == 11),
                                        perf_mode=DR, skip_group_check=True)
                                    n += 1
                        return go
                    def evac():
                        bias = (bq_sb, bk_sb)[tgt]
                        dst = (qT_r, kT_r)[tgt]
                        nc.vector.tensor_scalar(
                            dst[:, p2, ns0:ns0 + 512],
                            ps.rearrange("p a b -> p (a b)"),
                            bias[:, p2:p2 + 1], None, OP.add)
                    return [mk(0), mk(1), evac]

                def v_round(vr):
                    # two key-blocks (ms = 2*vr, 2*vr+1); psum [128, 2, 256]
                    ps = pqkv.tile([P, 2, 256], F32, name=f"v{c4}_{vr}", tag="pq")

                    def mk(g):
                        def go():
                            ms = 2 * vr + g
                            msl = slice(ms * P, (ms + 1) * P)
                            n = 0
                            for (wl, hl) in TERMS:
                                for ktp in range(KT // 2):
                                    ksl = slice(2 * ktp, 2 * ktp + 2)
                                    nc.tensor.matmul(
                                        ps[:, g, :],
                                        lhsT=ht[:, hl, ksl, msl],
                                        rhs=w8_r[:, wl, ksl, 2 * GD:3 * GD],
                                        start=(g == 0 and n == 0),
                                        stop=(g == 1 and n == 11),
                                        perf_mode=DR, skip_group_check=True)
                                    n += 1
                        return go
                    def evac():
                        jt = 4 * c4 + 2 * vr
                        nc.vector.tensor_copy(
                            v_r[:, jt:jt + 2, :, 0:64],
                            ps.rearrange("p g (hh fd) -> p g hh fd", fd=64))
                    return [mk(0), mk(1), evac]

                for p2 in range(PAIRS):
                    out.append(qk_round(0, p2))
                for p2 in range(PAIRS):
                    out.append(qk_round(1, p2))
                for vr in range(2):
                    out.append(v_round(vr))
                res = []
                for r in out:
                    a, b, ev = r
                    res.append(a)
                    # evac rides with the second half-closure
                    res.append(lambda b=b, ev=ev: (b(), ev()))
                return res

            def proj_closures(c4):
                """Output projection of chunk c4 (reads outT_r, writes out_d)."""
                res = []
                for rr in range(4):
                    r16 = 4 * c4 + rr
                    rsl = slice(r16 * P, (r16 + 1) * P)
                    ot = [None]

                    def half(n2, rr=rr, r16=r16, rsl=rsl, ot=ot):
                        def go():
                            nsl = slice(n2 * 512, (n2 + 1) * 512)
                            ps = pqkv.tile([P, 512], F32, name=f"pr{r16}_{n2}", tag="pq")
                            for p2 in range(PAIRS):
                                nc.tensor.matmul(
                                    ps, lhsT=outT_r[:, p2, rsl],
                                    rhs=wo_r[:, p2, nsl],
                                    start=(p2 == 0), stop=(p2 == PAIRS - 1))
                            if n2 == 0:
                                ot[0] = osb.tile([P, D], BF16, name=f"os{r16}", tag="os")
                            nc.vector.tensor_copy(ot[0][:, nsl], ps)
                            if n2 == 1:
                                nc.sync.dma_start(out_d[rsl, :], ot[0])
                        return go
                    res.append(half(0))
                    res.append(half(1))
                return res

            # ---------------- attention ----------------
            def attn_p2(p2, c4):
                njt = 4 * c4 + 4
                isl0 = c4 * 512
                pv = [ppv.tile([P, 2, 2, 65], F32, name=f"pv{p2}{c4}a", tag="pv"),
                      ppv.tile([P, 2, 2, 65], F32, name=f"pv{p2}{c4}b", tag="pv")]
                started = [False, False]
                tsp = [None, None]

                def scores_step(jt):
                    i0 = max(0, jt - 4 * c4) * P
                    jsl = slice(jt * P, (jt + 1) * P)
                    isw = slice(isl0 + i0, isl0 + 512)
                    ps_s = pss.tile([P, 1024], F32, name=f"s{p2}{c4}_{jt % 2}", tag="s")
                    for h in range(2):
                        nc.tensor.matmul(
                            ps_s[:, 512 * h + i0:512 * h + 512],
                            lhsT=kT_r[64 * h:64 * h + 64, p2, jsl],
                            rhs=qT_r[64 * h:64 * h + 64, p2, isw],
                            start=True, stop=True)
                    E = epool.tile([P, 1024], BF16, name=f"E{jt % 3}", tag="E")
                    if i0 == 0:
                        nc.scalar.activation(E, ps_s, AF.Exp, scale=SCALE / (WSCALE * WSCALE))
                    else:
                        ev = E.rearrange("p (h n) -> p h n", h=2)[:, :, i0:512]
                        sv = ps_s.rearrange("p (h n) -> p h n", h=2)[:, :, i0:512]
                        nc.scalar.activation(ev, sv, AF.Exp, scale=SCALE / (WSCALE * WSCALE))
                    if jt >= 4 * c4:
                        mv = E.rearrange("p (h n) -> p h n", h=2)[:, :, i0:i0 + P]
                        nc.gpsimd.tensor_tensor(
                            mv, mv,
                            masks_r[:, None, :].to_broadcast((P, 2, P)), OP.mult)
                    return E

                def pv_step(jt, E):
                    def go():
                        qs0 = max(0, jt - 4 * c4)
                        for qs in range(qs0, 4):
                            bk = qs // 2
                            for h in range(2):
                                st = not started[bk]
                                started[bk] = True
                                nc.tensor.matmul(
                                    pv[bk][:, qs % 2, h, :],
                                    lhsT=E[:, 512 * h + 128 * qs:512 * h + 128 * qs + 128],
                                    rhs=v_r[:, jt, 2 * p2 + h, :],
                                    start=st,
                                    stop=(jt == 4 * c4 + (2 * bk + 1) and qs == 2 * bk + 1 and h == 1),
                                    perf_mode=None, skip_group_check=True)
                    return go

                def end_bank(bk):
                    def go():
                        rec = rpool.tile([P, 2, 2, 1], F32, name=f"r{p2}{bk}", tag="r")
                        nc.vector.reciprocal(rec, pv[bk][:, :, :, 64:65])
                        at = apool.tile([P, 2, 2, 64], BF16, name=f"a{p2}{bk}", tag="a")
                        nc.vector.tensor_tensor(
                            at, pv[bk][:, :, :, 0:64],
                            rec.to_broadcast((P, 2, 2, 64)), OP.mult)
                        return at
                    return go

                def transp_bank(bk, at):
                    def go():
                        t = ppv.tile([P, 2, P], BF16, name=f"t{p2}{bk}", tag="pv")
                        tsp[bk] = t
                        for q in range(2):
                            nc.tensor.transpose(
                                t[:, q, :], at[:, q, :, :].rearrange("p h f -> p (h f)"),
                                ident_r)
                    return go

                def evac2_bank(bk):
                    def go():
                        qsl = slice(isl0 + 256 * bk, isl0 + 256 * (bk + 1))
                        nc.vector.tensor_scalar(
                            outT_r[:, p2, qsl],
                            tsp[bk].rearrange("p a b -> p (a b)"),
                            bv_sb[:, p2:p2 + 1], None, OP.add)
                    return go

                pend_pv = None
                post = collections.deque()  # delayed endgame closures
                for jt in range(njt):
                    E = scores_step(jt)
                    pstep()
                    if post:
                        post.popleft()()
                    if pend_pv is not None:
                        pend_pv()
                        if jt == 4 * c4 + 2:
                            # bank0 stopped at jt-1's pv (just emitted)
                            atA = end_bank(0)()
                            post.append(transp_bank(0, atA))
                            post.append(evac2_bank(0))
                    pend_pv = pv_step(jt, E)
                pend_pv()
                atB = end_bank(1)()
                while post:
                    pstep()
                    post.popleft()()
                pstep()
                transp_bank(1, atB)()
                pstep()
                evac2_bank(1)()

            # ---------------- main pipeline ----------------
            for cl in qkv_closures(0):
                cl()
            for c4 in range(NCH):
                if c4 + 1 < NCH:
                    filler.extend(qkv_closures(c4 + 1))
                if c4 + 2 < NCH:
                    prefetch_h(c4 + 2)
                njt = 4 * c4 + 4
                pace_chunk(2 * (njt + 3))
                attn_p2(0, c4)
                attn_p2(1, c4)
                drain(len(filler))  # qkv(c4+1) must land before attn(c4+1)
                filler.extend(proj_closures(c4))
            while filler:
                filler.popleft()()

            if DBG:
                nc.sync.dma_start(dbg_q[:, :, :], qT_r)
                nc.sync.dma_start(dbg_k[:, :, :], kT_r)
                nc.sync.dma_start(dbg_v[:, :, :, :], v_r)
                nc.sync.dma_start(dbg_o[:, :, :], outT_r)

    import bass_rust as _br
    _br.move_matmul_waits_to_ldweights(nc.m)
    _br.generate_event_semaphores(nc)
    nc.finalize()
    return nc


E4 = ml_dtypes.float8_e4m3
BF = ml_dtypes.bfloat16


def _split8(x):
    hi = x.astype(E4)
    lo = (x - hi.astype(np.float32)).astype(E4)
    return np.stack([hi, lo])


def _prep_inputs(hidden_states, Wqkv, bqkv, Wo):
    j = np.arange(P)[:, None]
    i = np.arange(P)[None, :]
    masks = (i >= j).astype(BF)
    ident = np.eye(P, dtype=np.float32).astype(BF)
    in_maps = []
    h8_cache = {}
    for c in range(8):
        b, g = c // 4, c % 4
        if b not in h8_cache:
            h8_cache[b] = _split8(np.ascontiguousarray(hidden_states[b].T))
        qs = slice(g * GD, (g + 1) * GD)
        wq = Wqkv[:, qs]
        wk = Wqkv[:, D + g * GD:D + (g + 1) * GD]
        wv = Wqkv[:, 2 * D + g * GD:2 * D + (g + 1) * GD]
        w_cat = np.ascontiguousarray(np.concatenate([wq, wk, wv], axis=1)) * np.float32(WSCALE)
        bqkv_c = np.ascontiguousarray(np.concatenate(
            [bqkv[qs], bqkv[D + g * GD:D + (g + 1) * GD],
             bqkv[2 * D + g * GD:2 * D + (g + 1) * GD]])).astype(np.float32) * np.float32(WSCALE)
        wo_c = np.ascontiguousarray(Wo[g * GD:(g + 1) * GD, :] / WSCALE).astype(BF)
        in_maps.append({
            "h8": h8_cache[b], "w8": _split8(w_cat), "wo": wo_c,
            "bqkv": bqkv_c, "masks": masks, "ident": ident,
        })
    return in_maps


_last_results = None


def kernel(hidden_states, attention_mask, Wqkv, bqkv, Wo, bo):
    """Full-input, full-output causal self-attention on 8 NeuronCores."""
    global _last_results
    from concourse.bass_utils import run_bass_kernel_spmd

    hidden_states = np.asarray(hidden_states, dtype=np.float32)
    Wqkv = np.asarray(Wqkv, dtype=np.float32)
    bqkv = np.asarray(bqkv, dtype=np.float32)
    Wo = np.asarray(Wo, dtype=np.float32)
    bo = np.asarray(bo, dtype=np.float32)

    if "nc" not in _nc_cache:
        _nc_cache["nc"] = build_nc()
    nc = _nc_cache["nc"]

    in_maps = _prep_inputs(hidden_states, Wqkv, bqkv, Wo)
    res = run_bass_kernel_spmd(nc, in_maps, core_ids=list(range(8)))
    _last_results = res

    parts = [r["partial"].astype(np.float32) for r in res.results]
    out = np.empty((B, C, D), dtype=np.float32)
    for b in range(B):
        acc = parts[4 * b].astype(np.float64)
        for g in range(1, 4):
            acc = acc + parts[4 * b + g]
        out[b] = (acc + bo.astype(np.float64)).astype(np.float32)
    return out


# revision 24
# speedup vs baseline: 1.5012x; 1.0963x over previous
"""Causal self-attention Trainium2 kernel (8 NeuronCores, batch x head-group sharded).

Problem: hidden [2, 2048, 1024], 16 heads x 64 dim, causal softmax attention,
QKV projection + output projection, all fp32 in/out.

Sharding: core c handles batch b = c//4 and head group g = c%4 (4 heads).
Each core computes qkv projections for its 4 heads, causal attention, and a
partial output projection (row-shard of Wo). Host sums the 4 partials per
batch and adds bo.

Numerics / engine strategy:
  - QKV projection runs as fp8e4m3 DoubleRow matmuls on host-pre-split
    operands (x ~= hi + lo, both e4m3; products hi*hi + hi*lo + lo*hi).
    Error ~2^-7 relative, at 2.67x the fp32r matmul rate.
  - q/k/v/E and the attention output are bf16; scores accumulate in fp32
    PSUM; softmax denominators ride the PV matmul as a 65th "ones" column.
  - PV uses the q-stationary orientation: out[q, f] = sum_j E[j,q] v[j,f],
    so each 128-key block costs 65 PE rows instead of ~1024, and the
    softmax denominator lands per-partition (reciprocal is one tiny DVE op,
    broadcast along free). A bf16 PE transpose (identity moving operand)
    restores the feature-major layout for the output projection.
  - The emission schedule software-pipelines: scores(jt+1) and QKV/proj
    "filler" matmuls execute while ACT runs exp(jt); PV(jt) lands after.
"""
import collections
import math
import os
import re as _re

import numpy as np
import ml_dtypes

import concourse.bass as bass
import concourse.mybir as mybir
import concourse.tile as tile

# ---------------------------------------------------------------------------
# The final TileContext drain carries one wait per proc (the Drain instruction
# has a single sync-wait slot in this walrus build).
from concourse.vector_clock import ScopedClock as _ScopedClock, VectorClock as _VectorClock


def _split_drain_and_barrier(self, tick_clock, wait_clock):
    nc = self.nc
    vals = [int(x) for x in _re.findall(r"\d+", repr(tick_clock.global_clock))]
    procs = [(i, v) for i, v in enumerate(vals) if v > 0]
    for idx, val in procs:
        vc = _VectorClock([0] * len(vals))
        vc.require_at_least(idx, val)
        d = nc.sync.drain()
        wait_clock.add_sem_waits(d.ins, _ScopedClock({None: vc}))
    nc.all_engine_barrier()
    popped = nc._tile_sem_poison_stack.pop()
    assert popped is self._sem_poison
    nc.clear_and_free_semaphores(list(self.sems.allocated().values()))
    nc.all_engine_barrier()


tile.TileContext._drain_and_barrier = _split_drain_and_barrier
# ---------------------------------------------------------------------------

F32 = mybir.dt.float32
BF16 = mybir.dt.bfloat16
F8 = mybir.dt.float8e4
AF = mybir.ActivationFunctionType
OP = mybir.AluOpType
DR = mybir.MatmulPerfMode.DoubleRow

B, C, D = 2, 2048, 1024
H, HD = 16, 64
SCALE = HD ** -0.5  # 0.125
WSCALE = 64.0        # host scales Wqkv by this before fp8 split (fp8 subnormal
                     # underflow otherwise: sigma_W ~ 0.03); q/k carry x64 each
                     # -> exp scale absorbs 1/WSCALE^2; v's x64 folds into Wo
P = 128
KT = D // P          # 8 contraction tiles of 128
NCH = C // 512       # 4 seq chunks of 512
HPC = 4              # heads per core
PAIRS = 2            # head pairs per core
GD = HPC * HD        # 256 features per core per tensor

_nc_cache = {}


def build_nc():
    nc = bass.Bass("TRN2")

    h8_d = nc.dram_tensor("h8", [2, D, C], F8, kind="ExternalInput")      # [hl, d, n]
    w8_d = nc.dram_tensor("w8", [2, D, 3 * GD], F8, kind="ExternalInput") # [hl, d, m]
    wo_d = nc.dram_tensor("wo", [GD, D], BF16, kind="ExternalInput")
    bqkv_d = nc.dram_tensor("bqkv", [3 * GD], F32, kind="ExternalInput")
    masks_d = nc.dram_tensor("masks", [P, P], BF16, kind="ExternalInput")
    ident_d = nc.dram_tensor("ident", [P, P], BF16, kind="ExternalInput")
    out_d = nc.dram_tensor("partial", [C, D], BF16, kind="ExternalOutput")
    DBG = os.environ.get("KERNEL_DEBUG") == "1"
    if DBG:
        dbg_q = nc.dram_tensor("dbg_q", [P, PAIRS, C], BF16, kind="ExternalOutput")
        dbg_k = nc.dram_tensor("dbg_k", [P, PAIRS, C], BF16, kind="ExternalOutput")
        dbg_v = nc.dram_tensor("dbg_v", [P, C // P, HPC, 65], BF16, kind="ExternalOutput")
        dbg_o = nc.dram_tensor("dbg_o", [P, PAIRS, C], BF16, kind="ExternalOutput")

    h8_v = h8_d[:, :, :].rearrange("hl (kt p) n -> p hl kt n", p=P)   # [128, 2, 8, 2048]
    w8_v = w8_d[:, :, :].rearrange("hl (kt p) m -> p hl kt m", p=P)   # [128, 2, 8, 768]
    wo_v = wo_d[:, :].rearrange("(p2 p) n -> p p2 n", p=P)            # [128, 2, 1024]

    with tile.TileContext(nc) as tc:
        import contextlib

        with contextlib.ExitStack() as ctx:
            persist = ctx.enter_context(tc.tile_pool(name="persist", bufs=1))
            h8p = ctx.enter_context(tc.tile_pool(name="h8p", bufs=4))
            epool = ctx.enter_context(tc.tile_pool(name="epool", bufs=6))
            apool = ctx.enter_context(tc.tile_pool(name="apool", bufs=3))
            rpool = ctx.enter_context(tc.tile_pool(name="rpool", bufs=3))
            osb = ctx.enter_context(tc.tile_pool(name="osb", bufs=4))
            pqkv = ctx.enter_context(tc.tile_pool(name="pqkv", bufs=2, space="PSUM"))
            pss = ctx.enter_context(tc.tile_pool(name="pss", bufs=2, space="PSUM"))
            ppv = ctx.enter_context(tc.tile_pool(name="ppv", bufs=2, space="PSUM"))

            # ---------------- persistent tiles ----------------
            w8_r = persist.tile([P, 2, KT, 3 * GD], F8)        # 12KB/part
            wo_r = persist.tile([P, PAIRS, D], BF16)           # 4KB
            masks_r = persist.tile([P, P], BF16)
            ident_r = persist.tile([P, P], BF16)
            qT_r = persist.tile([P, PAIRS, C], BF16)           # 8KB
            kT_r = persist.tile([P, PAIRS, C], BF16)           # 8KB
            # v per key-block jt, head hh (= 2*p2+h): [v(64) | 1]
            v_r = persist.tile([P, C // P, HPC, 65], BF16)     # 8.3KB
            outT_r = persist.tile([P, PAIRS, C], BF16)         # 8KB
            smalls = persist.tile([P, 6], F32)
            bq_sb = smalls[:, 0:2]
            bk_sb = smalls[:, 2:4]
            bv_sb = smalls[:, 4:6]

            # ---------------- one-time loads ----------------
            # Critical-path data first: the first hi*hi matmul needs w8-hi
            # (Pool SWDGE) and h8(0)-hi (SP HWDGE, queued ahead of the small
            # config loads). Everything else streams behind.
            h8t = {}

            def prefetch_h(c4, eng=None):
                t = h8p.tile([P, 2, KT, 512], F8, name=f"h8_{c4}", tag="h8")
                ns = slice(c4 * 512, (c4 + 1) * 512)
                for hl in range(2):
                    e = eng[hl] if eng else nc.gpsimd
                    e.dma_start(t[:, hl, :, :], h8_v[:, hl, :, ns])
                h8t[c4] = t

            nc.scalar.dma_start(w8_r[:, 0, :, GD:2 * GD], w8_v[:, 0, :, GD:2 * GD])
            prefetch_h(0, eng=(nc.sync, nc.sync))
            nc.gpsimd.dma_start(w8_r[:, 0, :, 0:GD], w8_v[:, 0, :, 0:GD])
            nc.sync.dma_start(bq_sb, bqkv_d[0:GD].rearrange("(p2 d) -> d p2", d=P))
            nc.sync.dma_start(bk_sb, bqkv_d[GD:2 * GD].rearrange("(p2 d) -> d p2", d=P))
            nc.scalar.dma_start(w8_r[:, 1, :, 0:2 * GD], w8_v[:, 1, :, 0:2 * GD])
            nc.gpsimd.dma_start(w8_r[:, 0, :, 2 * GD:], w8_v[:, 0, :, 2 * GD:])
            nc.scalar.dma_start(w8_r[:, 1, :, 2 * GD:], w8_v[:, 1, :, 2 * GD:])
            nc.sync.dma_start(bv_sb, bqkv_d[2 * GD:3 * GD].rearrange("(p2 d) -> d p2", d=P))
            nc.sync.dma_start(masks_r, masks_d[:, :])
            nc.sync.dma_start(wo_r[:, :, :], wo_v[:, :, :])
            nc.sync.dma_start(ident_r, ident_d[:, :])
            nc.vector.memset(v_r[:, :, :, 64:65], 1.0)
            prefetch_h(1, eng=(nc.scalar, nc.gpsimd))

            # ---------------- work-closure machinery ----------------
            filler = collections.deque()

            def drain(n):
                for _ in range(min(n, len(filler))):
                    filler.popleft()()

            pace = {"total": 0, "step": 0, "items": 0, "done": 0}

            def pace_chunk(total_steps):
                pace.update(total=total_steps, step=0, items=len(filler), done=0)

            def pstep():
                pace["step"] += 1
                tgt = -(-pace["items"] * pace["step"] // pace["total"])  # ceil
                k = min(tgt - pace["done"], len(filler))
                if k > 0:
                    pace["done"] += k
                    drain(k)

            # fp8 DoubleRow split-matmul term list: (w_hl, h_hl)
            TERMS = ((0, 0), (0, 1), (1, 0))

            def qkv_closures(c4):
                """Emit-closures for the qkv projection of chunk c4."""
                ns0 = c4 * 512
                ht = h8t[c4]
                out = []

                def qk_round(tgt, p2):
                    # tgt: 0=q, 1=k ; psum [128, 2, 256] = two 256-seq subtiles
                    fsl = slice(tgt * GD + p2 * P, tgt * GD + (p2 + 1) * P)
                    ps = pqkv.tile([P, 2, 256], F32, name=f"qk{c4}_{tgt}{p2}", tag="pq")

                    def mk(sub):
                        def go():
                            nsub = slice(sub * 256, (sub + 1) * 256)
                            first = sub == 0
                            n = 0
                            for (wl, hl) in TERMS:
                                for ktp in range(KT // 2):
                                    ksl = slice(2 * ktp, 2 * ktp + 2)
                                    nc.tensor.matmul(
                                        ps[:, sub, :],
                                        lhsT=w8_r[:, wl, ksl, fsl],
                                        rhs=ht[:, hl, ksl, nsub],
                                        start=(first and n == 0),
                                        stop=(not first and n == 11),
                                        perf_mode=DR, skip_group_check=True)
                                    n += 1
                        return go
                    def evac():
                        bias = (bq_sb, bk_sb)[tgt]
                        dst = (qT_r, kT_r)[tgt]
                        if c4 <= 1:
                            # ACT is idle in early chunks; keep DVE free
                            nc.scalar.activation(
                                dst[:, p2, ns0:ns0 + 512],
                                ps.rearrange("p a b -> p (a b)"),
                                AF.Copy, bias=bias[:, p2:p2 + 1])
                        else:
                            nc.vector.tensor_scalar(
                                dst[:, p2, ns0:ns0 + 512],
                                ps.rearrange("p a b -> p (a b)"),
                                bias[:, p2:p2 + 1], None, OP.add)
                    return [mk(0), mk(1), evac]

                def v_round(vr):
                    # two key-blocks (ms = 2*vr, 2*vr+1); psum [128, 2, 256]
                    ps = pqkv.tile([P, 2, 256], F32, name=f"v{c4}_{vr}", tag="pq")

                    def mk(g):
                        def go():
                            ms = 2 * vr + g
                            msl = slice(ms * P, (ms + 1) * P)
                            n = 0
                            for (wl, hl) in TERMS:
                                for ktp in range(KT // 2):
                                    ksl = slice(2 * ktp, 2 * ktp + 2)
                                    nc.tensor.matmul(
                                        ps[:, g, :],
                                        lhsT=ht[:, hl, ksl, msl],
                                        rhs=w8_r[:, wl, ksl, 2 * GD:3 * GD],
                                        start=(g == 0 and n == 0),
                                        stop=(g == 1 and n == 11),
                                        perf_mode=DR, skip_group_check=True)
                                    n += 1
                        return go
                    def evac():
                        jt = 4 * c4 + 2 * vr
                        nc.vector.tensor_copy(
                            v_r[:, jt:jt + 2, :, 0:64],
                            ps.rearrange("p g (hh fd) -> p g hh fd", fd=64))
                    return [mk(0), mk(1), evac]

                out.append(qk_round(1, 0))
                out.append(qk_round(0, 0))
                out.append(v_round(0))
                out.append(v_round(1))
                out.append(qk_round(0, 1))
                out.append(qk_round(1, 1))
                res = []
                for r in out:
                    a, b, ev = r
                    res.append(a)
                    # evac rides with the second half-closure
                    res.append(lambda b=b, ev=ev: (b(), ev()))
                return res

            def proj_closures(c4):
                """Output projection of chunk c4 (reads outT_r, writes out_d)."""
                res = []
                for rr in range(4):
                    r16 = 4 * c4 + rr
                    rsl = slice(r16 * P, (r16 + 1) * P)
                    ot = [None]

                    def half(n2, rr=rr, r16=r16, rsl=rsl, ot=ot):
                        def go():
                            nsl = slice(n2 * 512, (n2 + 1) * 512)
                            ps = pqkv.tile([P, 512], F32, name=f"pr{r16}_{n2}", tag="pq")
                            for p2 in range(PAIRS):
                                nc.tensor.matmul(
                                    ps, lhsT=outT_r[:, p2, rsl],
                                    rhs=wo_r[:, p2, nsl],
                                    start=(p2 == 0), stop=(p2 == PAIRS - 1))
                            if n2 == 0:
                                ot[0] = osb.tile([P, D], BF16, name=f"os{r16}", tag="os")
                            nc.vector.tensor_copy(ot[0][:, nsl], ps)
                            if n2 == 1:
                                nc.sync.dma_start(out_d[rsl, :], ot[0])
                        return go
                    res.append(half(0))
                    res.append(half(1))
                return res

            # ---------------- attention ----------------
            post = collections.deque()  # endgame closures, spill across p2/chunks

            def attn_p2(p2, c4):
                njt = 4 * c4 + 4
                isl0 = c4 * 512
                # allocation order fixes psum slot rotation: pvA->s0, pvB->s1,
                # tA->s0 (waits evac1-A), tB->s1 (waits evac1-B)
                pv = [ppv.tile([P, 2, 2, 65], F32, name=f"pv{p2}{c4}a", tag="pv"),
                      ppv.tile([P, 2, 2, 65], F32, name=f"pv{p2}{c4}b", tag="pv")]
                tsp = [ppv.tile([P, 2, P], BF16, name=f"t{p2}{c4}a", tag="pv"),
                       ppv.tile([P, 2, P], BF16, name=f"t{p2}{c4}b", tag="pv")]
                started = [False, False]

                def scores_step(jt):
                    i0 = max(0, jt - 4 * c4) * P
                    jsl = slice(jt * P, (jt + 1) * P)
                    isw = slice(isl0 + i0, isl0 + 512)
                    ps_s = pss.tile([P, 1024], F32, name=f"s{p2}{c4}_{jt % 2}", tag="s")
                    for h in range(2):
                        nc.tensor.matmul(
                            ps_s[:, 512 * h + i0:512 * h + 512],
                            lhsT=kT_r[64 * h:64 * h + 64, p2, jsl],
                            rhs=qT_r[64 * h:64 * h + 64, p2, isw],
                            start=True, stop=True)
                    E = epool.tile([P, 1024], BF16, name=f"E{jt % 3}", tag="E")
                    if i0 == 0:
                        nc.scalar.activation(E, ps_s, AF.Exp, scale=SCALE / (WSCALE * WSCALE))
                    else:
                        ev = E.rearrange("p (h n) -> p h n", h=2)[:, :, i0:512]
                        sv = ps_s.rearrange("p (h n) -> p h n", h=2)[:, :, i0:512]
                        nc.scalar.activation(ev, sv, AF.Exp, scale=SCALE / (WSCALE * WSCALE))
                    if jt >= 4 * c4:
                        mv = E.rearrange("p (h n) -> p h n", h=2)[:, :, i0:i0 + P]
                        nc.vector.tensor_tensor(
                            mv, mv,
                            masks_r[:, None, :].to_broadcast((P, 2, P)), OP.mult)
                    return E

                def pv_step(jt, E):
                    def go():
                        qs0 = max(0, jt - 4 * c4)
                        for qs in range(qs0, 4):
                            bk = qs // 2
                            for h in range(2):
                                st = not started[bk]
                                started[bk] = True
                                nc.tensor.matmul(
                                    pv[bk][:, qs % 2, h, :],
                                    lhsT=E[:, 512 * h + 128 * qs:512 * h + 128 * qs + 128],
                                    rhs=v_r[:, jt, 2 * p2 + h, :],
                                    start=st,
                                    stop=(jt == 4 * c4 + (2 * bk + 1) and qs == 2 * bk + 1 and h == 1),
                                    perf_mode=None, skip_group_check=True)
                    return go

                def end_bank(bk):
                    def go():
                        rec = rpool.tile([P, 2, 2, 1], F32, name=f"r{p2}{bk}", tag="r")
                        nc.vector.reciprocal(rec, pv[bk][:, :, :, 64:65])
                        at = apool.tile([P, 2, 2, 64], BF16, name=f"a{p2}{bk}", tag="a")
                        nc.vector.tensor_tensor(
                            at, pv[bk][:, :, :, 0:64],
                            rec.to_broadcast((P, 2, 2, 64)), OP.mult)
                        return at
                    return go

                def transp_bank(bk, at):
                    def go():
                        t = tsp[bk]
                        for q in range(2):
                            nc.tensor.transpose(
                                t[:, q, :], at[:, q, :, :].rearrange("p h f -> p (h f)"),
                                ident_r)
                    return go

                def evac2_bank(bk):
                    def go():
                        qsl = slice(isl0 + 256 * bk, isl0 + 256 * (bk + 1))
                        nc.vector.tensor_scalar(
                            outT_r[:, p2, qsl],
                            tsp[bk].rearrange("p a b -> p (a b)"),
                            bv_sb[:, p2:p2 + 1], None, OP.add)
                    return go

                pend_pv = None
                for jt in range(njt):
                    E = scores_step(jt)
                    pstep()
                    if post:
                        post.popleft()()
                    if pend_pv is not None:
                        pend_pv()
                        if jt == 4 * c4 + 2:
                            # bank0 stopped at jt-1's pv (just emitted)
                            atA = end_bank(0)()
                            post.append(transp_bank(0, atA))
                            post.append(evac2_bank(0))
                    pend_pv = pv_step(jt, E)
                pend_pv()
                atB = end_bank(1)()
                post.append(transp_bank(1, atB))
                post.append(evac2_bank(1))

            # ---------------- main pipeline ----------------
            qkv0 = qkv_closures(0)
            for cl in qkv0[:6]:
                cl()
            filler.extend(qkv0[6:])
            hold = collections.defaultdict(list)  # chunk -> delayed proj work
            for c4 in range(NCH):
                if c4 + 1 < NCH:
                    filler.extend(qkv_closures(c4 + 1))
                # proj(c4') rides two chunks later: late chunks have more exp
                # latency to hide and no qkv filler left
                filler.extend(hold.pop(c4, []))
                if c4 + 2 < NCH:
                    prefetch_h(c4 + 2)
                njt = 4 * c4 + 4
                pace_chunk(2 * njt)
                attn_p2(0, c4)
                attn_p2(1, c4)
                # boundary: flush endgame + any unfinished qkv(c4+1), interleaved
                while post or filler:
                    if post:
                        post.popleft()()
                    drain(1)
                if c4 < NCH - 1:
                    hold[min(c4 + 2, NCH - 1)].extend(proj_closures(c4))
            filler.extend(proj_closures(NCH - 1))
            while filler:
                filler.popleft()()

            if DBG:
                nc.sync.dma_start(dbg_q[:, :, :], qT_r)
                nc.sync.dma_start(dbg_k[:, :, :], kT_r)
                nc.sync.dma_start(dbg_v[:, :, :, :], v_r)
                nc.sync.dma_start(dbg_o[:, :, :], outT_r)

    import bass_rust as _br
    _br.move_matmul_waits_to_ldweights(nc.m)
    _br.generate_event_semaphores(nc)
    nc.finalize()
    return nc


E4 = ml_dtypes.float8_e4m3
BF = ml_dtypes.bfloat16


def _split8(x):
    hi = x.astype(E4)
    lo = (x - hi.astype(np.float32)).astype(E4)
    return np.stack([hi, lo])


def _prep_inputs(hidden_states, Wqkv, bqkv, Wo):
    j = np.arange(P)[:, None]
    i = np.arange(P)[None, :]
    masks = (i >= j).astype(BF)
    ident = np.eye(P, dtype=np.float32).astype(BF)
    in_maps = []
    h8_cache = {}
    for c in range(8):
        b, g = c // 4, c % 4
        if b not in h8_cache:
            h8_cache[b] = _split8(np.ascontiguousarray(hidden_states[b].T))
        qs = slice(g * GD, (g + 1) * GD)
        wq = Wqkv[:, qs]
        wk = Wqkv[:, D + g * GD:D + (g + 1) * GD]
        wv = Wqkv[:, 2 * D + g * GD:2 * D + (g + 1) * GD]
        w_cat = np.ascontiguousarray(np.concatenate([wq, wk, wv], axis=1)) * np.float32(WSCALE)
        bqkv_c = np.ascontiguousarray(np.concatenate(
            [bqkv[qs], bqkv[D + g * GD:D + (g + 1) * GD],
             bqkv[2 * D + g * GD:2 * D + (g + 1) * GD]])).astype(np.float32) * np.float32(WSCALE)
        wo_c = np.ascontiguousarray(Wo[g * GD:(g + 1) * GD, :] / WSCALE).astype(BF)
        in_maps.append({
            "h8": h8_cache[b], "w8": _split8(w_cat), "wo": wo_c,
            "bqkv": bqkv_c, "masks": masks, "ident": ident,
        })
    return in_maps


_last_results = None


def kernel(hidden_states, attention_mask, Wqkv, bqkv, Wo, bo):
    """Full-input, full-output causal self-attention on 8 NeuronCores."""
    global _last_results
    from concourse.bass_utils import run_bass_kernel_spmd

    hidden_states = np.asarray(hidden_states, dtype=np.float32)
    Wqkv = np.asarray(Wqkv, dtype=np.float32)
    bqkv = np.asarray(bqkv, dtype=np.float32)
    Wo = np.asarray(Wo, dtype=np.float32)
    bo = np.asarray(bo, dtype=np.float32)

    if "nc" not in _nc_cache:
        _nc_cache["nc"] = build_nc()
    nc = _nc_cache["nc"]

    in_maps = _prep_inputs(hidden_states, Wqkv, bqkv, Wo)
    res = run_bass_kernel_spmd(nc, in_maps, core_ids=list(range(8)))
    _last_results = res

    parts = [r["partial"].astype(np.float32) for r in res.results]
    out = np.empty((B, C, D), dtype=np.float32)
    for b in range(B):
        acc = parts[4 * b].astype(np.float64)
        for g in range(1, 4):
            acc = acc + parts[4 * b + g]
        out[b] = (acc + bo.astype(np.float64)).astype(np.float32)
    return out
